# revision 25
# baseline (speedup 1.0000x reference)
"""RWKV ChannelMixer (single-token) on 8 Trainium2 NeuronCores.

Math (reference):
    xn  = LayerNorm(x) * ln_w + ln_b
    xk  = xn*tmk + prev*(1-tmk);  xr = xn*tmr + prev*(1-tmr)
    r   = sigmoid(rw @ xr)                       # (D,)
    k   = relu(kw @ xk)^2                        # (F,)
    out = x + r * (vw @ k)                       # (D,)
    returns (out, xn)

Sharding (8 cores, no collectives -- cross-core sync costs ~60us here):
    kw: F-row-sharded (512 rows/core)  -> local k chunk (512,)
    vw: F-col-sharded (512 cols/core)  -> partial v_i = vw[:,Fi] @ k_i (1024,)
    rw: D-row-sharded (128 rows/core)  -> r chunk (128,)
    LN/mix replicated.  Host unshard: v = sum_i v_i, r = concat(r_i),
    out = x + r*v.

Engines: dot-products run on the Vector engine (scalar_tensor_tensor
with accum_out = fused multiply + free-dim reduce, fp32 @ ~1 elem/lane/
cycle ~ 490GB/s > 358GB/s HBM/core).  TensorE does tiny selector-matmul
partition-broadcasts and output transposes.  Weights stream through
SBUF in natural row-major layout (host only slices/reshapes; pure
parameter products like tmk*lnw are folded on the host).
"""

import sys
import numpy as np

for _p in ("/opt/trn_rl_repo", "/root/.axon_site/_ro/trn_rl_repo"):
    if _p not in sys.path:
        sys.path.append(_p)

D = 1024
F = 4096
N_CORES = 8
FSH = F // N_CORES      # 512 kw rows / vw cols per core
DSH = D // N_CORES      # 128 rw rows per core
LN_EPS = 1e-5

_STATE = {}


def _body(nc, tc, mybir, stage):
    f32 = mybir.dt.float32
    Alu = mybir.AluOpType
    Act = mybir.ActivationFunctionType
    AxX = mybir.AxisListType.X

    kw_d = nc.dram_tensor("kw_p", [128, 4096], f32, kind="ExternalInput").ap()
    vw_d = nc.dram_tensor("vw_p", [128, 4096], f32, kind="ExternalInput").ap()
    rw_d = nc.dram_tensor("rw_p", [128, 1024], f32, kind="ExternalInput").ap()
    # stacked vectors [8, 10*128]: x, prev, ck=tmk*lnw, cr=tmr*lnw,
    # gk=tmk*lnb, gr=tmr*lnb, hk=1-tmk, hr=1-tmr, lnw, lnb
    sm_d = nc.dram_tensor("smalls", [8, 1280], f32, kind="ExternalInput").ap()

    xn_d = nc.dram_tensor("xn_out", [8, 128], f32, kind="ExternalOutput").ap()
    v_d = nc.dram_tensor("v_out", [8, 128], f32, kind="ExternalOutput").ap()
    r_d = nc.dram_tensor("r_out", [1, 128], f32, kind="ExternalOutput").ap()

    import contextlib
    with contextlib.ExitStack() as ctx:
        wp = ctx.enter_context(tc.tile_pool(name="w", bufs=1))
        vp = ctx.enter_context(tc.tile_pool(name="v", bufs=1))
        bp = ctx.enter_context(tc.tile_pool(name="bc", bufs=2, space="PSUM"))
        pp = ctx.enter_context(tc.tile_pool(name="ps", bufs=1, space="PSUM"))

        # ---- small packed DMA first, then bulk (same HWDGE FIFO: sm->kw->rw->vw)
        sm_sb = vp.tile([8, 1280], f32, tag="sm")
        nc.sync.dma_start(out=sm_sb[:], in_=sm_d[:])
        x_row = sm_sb[:, 0:128]
        pv_row = sm_sb[:, 128:256]
        ck = sm_sb[:, 256:384]
        cr = sm_sb[:, 384:512]
        gk_row = sm_sb[:, 512:640]
        gr_row = sm_sb[:, 640:768]
        hk_row = sm_sb[:, 768:896]
        hr_row = sm_sb[:, 896:1024]
        lw_row = sm_sb[:, 1024:1152]
        lb_row = sm_sb[:, 1152:1280]

        if stage >= 2:
            kw_sb = wp.tile([128, 4096], f32, tag="kw")
            rw_sb = wp.tile([128, 1024], f32, tag="rw")
            vw_sb = wp.tile([128, 4096], f32, tag="vw")
            for c in range(4):
                nc.sync.dma_start(out=kw_sb[:, c * 1024:(c + 1) * 1024],
                                  in_=kw_d[:, c * 1024:(c + 1) * 1024])
            nc.sync.dma_start(out=rw_sb[:], in_=rw_d[:])
            for c in range(4):
                nc.sync.dma_start(out=vw_sb[:, c * 1024:(c + 1) * 1024],
                                  in_=vw_d[:, c * 1024:(c + 1) * 1024])

        # ---- constants
        ones_c8 = vp.tile([8, 1], f32, tag="ones_c8")
        ones_r8 = vp.tile([1, 8], f32, tag="ones_r8")
        eps_t = vp.tile([1, 1], f32, tag="eps")
        eps8 = vp.tile([8, 1], f32, tag="eps8")
        nc.vector.memset(ones_c8[:], 1.0)
        nc.vector.memset(ones_r8[:], 1.0)
        nc.vector.memset(eps_t[:], LN_EPS)
        nc.vector.memset(eps8[:], LN_EPS)
        warm_sqrt = vp.tile([1, 1], f32, tag="warm_sqrt")
        nc.scalar.activation(warm_sqrt[:], eps_t[:], Act.Sqrt)
        if stage >= 3:
            # one-hot row-selector matrices (lhsT for row-broadcast matmuls)
            sel8 = vp.tile([8, 1024], f32, tag="sel8")
            sel2 = vp.tile([2, 256], f32, tag="sel2")
            nc.gpsimd.memset(sel8[:], 0.0)
            nc.gpsimd.memset(sel2[:], 0.0)
            nc.gpsimd.affine_select(
                out=sel8[:].rearrange("p (j q) -> p j q", j=8),
                in_=sel8[:].rearrange("p (j q) -> p j q", j=8),
                compare_op=Alu.not_equal, fill=1.0, base=0,
                pattern=[[-1, 8], [0, 128]], channel_multiplier=1)
            nc.gpsimd.affine_select(
                out=sel2[:].rearrange("p (j q) -> p j q", j=2),
                in_=sel2[:].rearrange("p (j q) -> p j q", j=2),
                compare_op=Alu.not_equal, fill=1.0, base=0,
                pattern=[[-1, 2], [0, 128]], channel_multiplier=1)

        from concourse.masks import make_identity
        ident = vp.tile([128, 128], f32, tag="ident")
        make_identity(nc, ident)

        # ---- LayerNorm stats over 1024 elems laid out [8, 128]
        s2 = vp.tile([8, 2], f32, tag="s2")
        xsq = vp.tile([8, 128], f32, tag="xsq")
        nc.vector.tensor_reduce(out=s2[:, 0:1], in_=x_row, axis=AxX, op=Alu.add)
        nc.vector.scalar_tensor_tensor(out=xsq[:], in0=x_row, scalar=1.0,
                                       in1=x_row, op0=Alu.mult, op1=Alu.mult,
                                       accum_out=s2[:, 1:2])

        psum_s = pp.tile([1, 2], f32, tag="pmisc", bufs=2)
        nc.tensor.matmul(psum_s[:], ones_c8[:], s2[:], start=True, stop=True)
        ssum = vp.tile([1, 2], f32, tag="ssum")     # raw [sum_x, sum_x2]
        nc.scalar.copy(ssum[:], psum_s[:])
        psum_b = pp.tile([8, 2], f32, tag="pmisc", bufs=2)
        nc.tensor.matmul(psum_b[:], ones_r8[:], ssum[:], start=True, stop=True)
        bc8 = vp.tile([8, 2], f32, tag="bc8")       # per-part raw sums
        nc.scalar.mul(bc8[:], psum_b[:], 1.0 / D)   # [mean, E[x^2]] per part

        mean8 = bc8[:, 0:1]
        var8 = vp.tile([8, 1], f32, tag="var8")
        std8 = vp.tile([8, 1], f32, tag="std8")
        rstd8 = vp.tile([8, 1], f32, tag="rstd8")
        nc.vector.tensor_mul(var8[:], mean8, mean8)
        nc.vector.tensor_sub(var8[:], bc8[:, 1:2], var8[:])
        nc.scalar.activation(std8[:], var8[:], Act.Sqrt, bias=eps8[:])
        nc.vector.reciprocal(rstd8[:], std8[:])

        # Mix offsets Ek = gk + prev*hk (coefficients ck/gk/hk are pure
        # parameter products, folded on the host).  Emitted here so the
        # in-order DVE runs them inside the PE stats round-trip window.
        ek = vp.tile([8, 128], f32, tag="ek")
        er = vp.tile([8, 128], f32, tag="er")
        nc.vector.tensor_mul(ek[:], pv_row, hk_row)
        nc.vector.tensor_add(ek[:], ek[:], gk_row)
        nc.vector.tensor_mul(er[:], pv_row, hr_row)
        nc.vector.tensor_add(er[:], er[:], gr_row)

        xn_pre = vp.tile([8, 128], f32, tag="xn_pre")
        nc.vector.tensor_scalar(out=xn_pre[:], in0=x_row,
                                scalar1=mean8, scalar2=rstd8[:],
                                op0=Alu.subtract, op1=Alu.mult)

        # ---- token mixes straight from xn_pre (critical path)
        xk_row = vp.tile([8, 128], f32, tag="xk")
        xr_row = vp.tile([8, 128], f32, tag="xr")
        nc.vector.tensor_mul(xk_row[:], xn_pre[:], ck)
        nc.vector.tensor_add(xk_row[:], xk_row[:], ek[:])
        nc.vector.tensor_mul(xr_row[:], xn_pre[:], cr)
        nc.vector.tensor_add(xr_row[:], xr_row[:], er[:])

        # full xn only feeds the output (off the critical path)
        xn_row = vp.tile([8, 128], f32, tag="xn")
        nc.vector.tensor_mul(xn_row[:], xn_pre[:], lw_row)
        nc.vector.tensor_add(xn_row[:], xn_row[:], lb_row)
        nc.sync.dma_start(out=xn_d[:], in_=xn_row[:])

        if stage < 3:
            return

        # ---- broadcast xk across partitions: [8,128] -> [128, 1024]
        xk_bc = vp.tile([128, 1024], f32, tag="xk_bc")
        for j in range(8):
            pb = bp.tile([128, 128], f32, tag="pb", name=f"pbk{j}")
            nc.tensor.matmul(pb[:], sel8[:, j * 128:(j + 1) * 128], xk_row[:],
                             start=True, stop=True)
            nc.scalar.copy(xk_bc[:, j * 128:(j + 1) * 128], pb[:])
        # xr needs no broadcast: transpose it once, the rw dot uses
        # per-partition scalars against host-transposed rw
        xrT_ps = pp.tile([128, 8], f32, tag="pmisc", bufs=2)
        nc.tensor.transpose(xrT_ps[:], xr_row[:], ident[0:8, 0:8])
        xrT = vp.tile([128, 8], f32, tag="xrT")
        nc.scalar.copy(xrT[:], xrT_ps[:])

        if stage < 4:
            return

        # ---- stage A: k chunk = sqrelu(kw_i @ xk).  Contraction split in
        #      halves (h0 dots start after the first 4 broadcast pairs);
        #      the h1 round goes chunk-major with a per-chunk-pair epilogue
        #      so the k transpose/broadcast for chunks 0-1 overlaps the
        #      remaining dots.
        def kwdot(c, h):
            nc.vector.scalar_tensor_tensor(
                out=scratch[:, h * 512:(h + 1) * 512],
                in0=kw_sb[:, c * 1024 + h * 512: c * 1024 + (h + 1) * 512]
                    .rearrange("p (j q) -> p j q", q=128),
                scalar=1.0,
                in1=xk_bc[:, h * 512:(h + 1) * 512]
                    .rearrange("p (j q) -> p j q", q=128),
                op0=Alu.mult, op1=Alu.mult,
                accum_out=kh[:, 3 * c: 3 * c + 1])

        def kwdot_q(c, q):
            # 256-wide quarter of the first half (j-blocks 2q, 2q+1)
            nc.vector.scalar_tensor_tensor(
                out=scratch[:, q * 256:(q + 1) * 256],
                in0=kw_sb[:, c * 1024 + q * 256: c * 1024 + (q + 1) * 256]
                    .rearrange("p (j x) -> p j x", x=128),
                scalar=1.0,
                in1=xk_bc[:, q * 256:(q + 1) * 256]
                    .rearrange("p (j x) -> p j x", x=128),
                op0=Alu.mult, op1=Alu.mult,
                accum_out=kh[:, 3 * c + 1 + q: 3 * c + 2 + q])

        scratch = vp.tile([128, 1024], f32, tag="scratch")
        kh = vp.tile([128, 12], f32, tag="kh")      # [c, (h1, q0, q1)] partials
        k_sb = vp.tile([128, 4], f32, tag="k")
        v_sb = vp.tile([128, 8], f32, tag="v")
        ksq = vp.tile([128, 4], f32, tag="ksq")
        khv = kh[:].rearrange("p (c t) -> p c t", t=3)
        for c in range(4):
            kwdot_q(c, 0)
        for c in range(4):
            kwdot_q(c, 1)
        for g in range(2):
            kwdot(2 * g, 1)
            kwdot(2 * g + 1, 1)
            # epilogue for chunks {2g, 2g+1}: sum partials, relu (into
            # scratch to pin DVE ordering), square
            nc.vector.tensor_add(k_sb[:, 2 * g:2 * g + 2],
                                 khv[:, 2 * g:2 * g + 2, 1],
                                 khv[:, 2 * g:2 * g + 2, 2])
            nc.vector.tensor_add(k_sb[:, 2 * g:2 * g + 2],
                                 k_sb[:, 2 * g:2 * g + 2],
                                 khv[:, 2 * g:2 * g + 2, 0])
            nc.vector.tensor_scalar_max(scratch[:, 2 * g:2 * g + 2],
                                        k_sb[:, 2 * g:2 * g + 2], 0.0)
            nc.vector.tensor_mul(ksq[:, 2 * g:2 * g + 2],
                                 scratch[:, 2 * g:2 * g + 2],
                                 scratch[:, 2 * g:2 * g + 2])

        if stage < 5:
            return

        # ---- k broadcast, pipelined per chunk-group: transpose [128, 2]
        #      -> [2, 128], 2 selector matmuls into the shared PSUM bank
        k_bc = pp.tile([128, 512], f32, tag="kbc_ps", bufs=1)
        for g in range(2):
            kT_ps = pp.tile([2, 128], f32, tag="pmisc", bufs=2, name=f"kT_ps{g}")
            nc.tensor.transpose(kT_ps[:], ksq[:, 2 * g:2 * g + 2], ident[:])
            kT = vp.tile([2, 128], f32, tag=f"kT{g}", name=f"kT{g}")
            nc.scalar.copy(kT[:], kT_ps[:])
            for cc in range(2):
                c = 2 * g + cc
                nc.tensor.matmul(k_bc[:, c * 128:(c + 1) * 128],
                                 sel2[:, cc * 128:(cc + 1) * 128], kT[:],
                                 start=True, stop=True)
        if stage < 6:
            return

        # ---- r = sigmoid(rw @ xr) via per-partition-scalar accumulation
        #      over host-transposed rw (no xr broadcast needed)
        # accumulate in a scratch region so the WAW dep on the kw-dot
        # outputs keeps the in-order DVE from hoisting this ahead of the
        # k epilogue
        acc_r = scratch[:, 512:640]
        nc.vector.tensor_scalar_mul(acc_r, rw_sb[:, 0:128], xrT[:, 0:1])
        for j in range(1, 8):
            nc.vector.scalar_tensor_tensor(
                out=acc_r, in0=rw_sb[:, j * 128:(j + 1) * 128],
                scalar=xrT[:, j:j + 1], in1=acc_r,
                op0=Alu.mult, op1=Alu.add)
        ones_c128 = vp.tile([128, 1], f32, tag="ones_c128")
        nc.vector.memset(ones_c128[:], 1.0)
        pre_r_ps = pp.tile([1, 128], f32, tag="pmisc", bufs=2)
        nc.tensor.matmul(pre_r_ps[:], ones_c128[:], acc_r,
                         start=True, stop=True)
        r_row = vp.tile([1, 128], f32, tag="r_row")
        nc.scalar.activation(r_row[:], pre_r_ps[:], Act.Sigmoid)
        nc.sync.dma_start(out=r_d[:], in_=r_row[:])


        # ---- stage V: v partial, 8 d-chunks of [128, 512] x k_bc
        for m in range(8):
            nc.vector.scalar_tensor_tensor(
                out=scratch[:, 0:512], in0=vw_sb[:, m * 512:(m + 1) * 512],
                scalar=1.0, in1=k_bc[:],
                op0=Alu.mult, op1=Alu.mult, accum_out=v_sb[:, m:m + 1])

        # ---- v output in row form (contiguous DMA): transpose via PE,
        #      split in halves so the first hides under the last vw dots
        for hh in range(2):
            vT_ps = pp.tile([4, 128], f32, tag="pmisc", bufs=2,
                            name=f"vT_ps{hh}")
            nc.tensor.transpose(vT_ps[:], v_sb[:, hh * 4:(hh + 1) * 4], ident[:])
            vT = vp.tile([4, 128], f32, tag=f"vT{hh}", name=f"vT{hh}")
            nc.scalar.copy(vT[:], vT_ps[:])
            nc.sync.dma_start(out=v_d[hh * 4:(hh + 1) * 4, :], in_=vT[:])


def _build(stage=6):
    import concourse.bacc as bacc
    import concourse.tile as tile
    from concourse import mybir

    nc = bacc.Bacc("TRN2", target_bir_lowering=False, debug=False,
                   num_devices=N_CORES)
    with tile.TileContext(nc) as tc:
        _body(nc, tc, mybir, stage)
    nc.compile()
    return nc


def _prep_shared(kw, vw, rw):
    """Slice + reshape weights per core (rows onto 128 partitions)."""
    kw_p, vw_p, rw_p = [], [], []
    for i in range(N_CORES):
        A = kw[i * FSH:(i + 1) * FSH, :]                # (512, 1024) rows f
        A = A.reshape(4, 128, 1024).transpose(1, 0, 2)  # [p, c, d]
        kw_p.append(np.ascontiguousarray(A.reshape(128, 4096)))

        B = rw[i * DSH:(i + 1) * DSH, :].T              # (1024, 128) rows d_in
        B = B.reshape(8, 128, DSH).transpose(1, 0, 2)   # [p, j, d_out]
        rw_p.append(np.ascontiguousarray(B.reshape(128, 1024)))

        C = vw[:, i * FSH:(i + 1) * FSH]                # (1024, 512) rows d
        C = C.reshape(8, 128, FSH).transpose(1, 0, 2)   # [p, m, f]
        vw_p.append(np.ascontiguousarray(C.reshape(128, 4096)))
    return kw_p, vw_p, rw_p


def _prep_smalls(x, state, tmk, tmr, lnw, lnb):
    vecs = [x, state[0], tmk * lnw, tmr * lnw, tmk * lnb, tmr * lnb,
            1.0 - tmk, 1.0 - tmr, lnw, lnb]
    sm = np.stack([v.reshape(8, 128) for v in vecs], axis=1)
    return np.ascontiguousarray(sm.reshape(8, 1280))


def kernel(x, state, time_mix_k, time_mix_r, kw, vw, rw, ln_weight, ln_bias):
    from concourse import bass_utils

    x = np.asarray(x, dtype=np.float32)
    state = np.asarray(state, dtype=np.float32)
    kw = np.asarray(kw, dtype=np.float32)
    vw = np.asarray(vw, dtype=np.float32)
    rw = np.asarray(rw, dtype=np.float32)
    tmk = np.asarray(time_mix_k, dtype=np.float32)
    tmr = np.asarray(time_mix_r, dtype=np.float32)
    lnw = np.asarray(ln_weight, dtype=np.float32)
    lnb = np.asarray(ln_bias, dtype=np.float32)

    if "nc" not in _STATE:
        _STATE["nc"] = _build()
    nc = _STATE["nc"]

    kw_p, vw_p, rw_p = _prep_shared(kw, vw, rw)
    sm = _prep_smalls(x, state, tmk, tmr, lnw, lnb)

    in_maps = [{"kw_p": kw_p[i], "vw_p": vw_p[i], "rw_p": rw_p[i], "smalls": sm}
               for i in range(N_CORES)]

    res = bass_utils.run_bass_kernel_spmd(nc, in_maps, core_ids=list(range(N_CORES)))

    # unshard: v = sum of partials, r = concat of chunks
    v = np.zeros(D, dtype=np.float64)
    for i in range(N_CORES):
        v += res.results[i]["v_out"].reshape(D).astype(np.float64)
    r = np.concatenate([res.results[i]["r_out"].reshape(DSH)
                        for i in range(N_CORES)])
    out = x + r * v.astype(np.float32)
    xn = res.results[0]["xn_out"].reshape(D)
    return np.asarray(out, dtype=np.float32), np.asarray(xn, dtype=np.float32)
